# revision 27
# baseline (speedup 1.0000x reference)
"""Trainium2 Bass kernel for nn_Attention_39049842655427.

Multi-head attention (RoPE + hard mask + soft gate mask) over 8
NeuronCores: data-parallel over batch (2) x tensor-parallel over heads
(16 heads -> 4 per core).  Each core computes q/k/v projections for its
4 heads, the head-sharded attention, and a partial output projection
(wo row-sharded); the host sums the 4 partials per batch and adds bo.

Math notes (exact up to float rounding):
  reference:  e = exp(s)*hard ; a1 = e/sum(e) ; a2 = a1*soft
              attn = a2/(sum(a2)+1e-6) ; out = attn @ v
  identity:   attn = f / (F + 1e-6*E),  f = e*hard*soft,
              F = sum(f), E = sum(e*hard)
  kernel:     g = exp(s) * M2,  M2 = hard*(soft+1e-6)
              => sum(g) = F + 1e-6*E exactly; numerator uses g instead
              of f, an O(1e-6) perturbation of attn.
  bv folding: sum_kv attn = 1 exactly under the g/sum(g) form, so the
              v-bias contributes bv per output channel post-softmax;
              it is moved to the host as bo_eff = bo + wo @ bv.
All matmuls run in bf16 with fp32 PSUM accumulation.  Scores are
computed transposed (s[kv,q]) so attn@v needs no on-device transpose;
RoPE pairs are pre-permuted (evens then odds) inside each head's 128
rows of wq/wk so the rotation partner is a partition offset of 64
(applied via a small SBUF->SBUF DMA).  128x512 blocks of M2 that are
exactly zero are skipped entirely, and within kept blocks leading
all-zero column spans (in 128 steps) narrow the score/exp/mask/AV work
(exact, data-adaptive: read from the actual mask and baked into the
compiled program; a dense mask falls back to the all-keep program).

For a causal keep pattern the whole kernel is a single software-
pipelined loop over 512-row chunks: K/Q projections for chunk s4, then
the off-diagonal attention blocks of query-column s4 (their k/v chunks
are older), then the V projection of chunk s4, then the diagonal
attention blocks and the softmax normalization, then the output
projection of column s4-1.  This keeps the scalar engine's exp work
spread across the projection matmuls instead of piling up in a
separate attention phase.
"""

import math
import sys

import numpy as np
import ml_dtypes

if "/opt/trn_rl_repo" not in sys.path:
    sys.path.append("/opt/trn_rl_repo")

import concourse.bass as bass  # noqa: E402,F401
import concourse.tile as tile  # noqa: E402
from concourse import bacc, mybir  # noqa: E402
from concourse.bass_utils import run_bass_kernel_spmd  # noqa: E402

B, S, D, H, DK = 2, 2048, 2048, 16, 128
N_CORES = 8
HPC = 4          # heads per core
DSH = HPC * DK   # 512, d-shard per core

BF16 = ml_dtypes.bfloat16

_NC_CACHE = {}

WARMUP_MM = 14   # HAM warm-up matmuls bridging the startup DMA window


def build_bass(s_len=S, keep=None, scols=None):
    """Build the SPMD single-core program (same NEFF on all 8 cores)."""
    f32 = mybir.dt.float32
    bf16 = mybir.dt.bfloat16
    KC = D // 128          # contraction chunks for projections
    SQ = s_len // 512      # 512-wide q/s chunks
    NKV = s_len // 128     # 128-row kv chunks
    JQ = D // 512          # output-column chunks
    KP = KC // 4
    if keep is None:
        keep = tuple(tuple(True for _ in range(NKV)) for _ in range(SQ))
    if scols is None:
        scols = tuple(tuple(0 for _ in range(NKV)) for _ in range(SQ))
    # causal pattern: every kept kv block of column q4 lives in chunks
    # <= q4, which is what lets attention interleave into the
    # projection loop.
    causal = all(kv // 4 <= q4
                 for q4 in range(SQ) for kv in range(NKV) if keep[q4][kv])

    nc = bacc.Bacc("TRN2", target_bir_lowering=False, debug=False,
                   num_devices=N_CORES)

    xT = nc.dram_tensor("xT", [SQ, KP, 128, 4, 512], bf16, kind="ExternalInput").ap()
    wqT = nc.dram_tensor("wqT", [KP, 128, 4, DSH], bf16, kind="ExternalInput").ap()
    wkT = nc.dram_tensor("wkT", [KP, 128, 4, DSH], bf16, kind="ExternalInput").ap()
    wvT = nc.dram_tensor("wvT", [KP, 128, 4, DSH], bf16, kind="ExternalInput").ap()
    woT = nc.dram_tensor("woT", [DSH, D], bf16, kind="ExternalInput").ap()
    bqp = nc.dram_tensor("bqp", [128, HPC], f32, kind="ExternalInput").ap()
    bkp = nc.dram_tensor("bkp", [128, HPC], f32, kind="ExternalInput").ap()
    cosp = nc.dram_tensor("cosp", [128, s_len], bf16, kind="ExternalInput").ap()
    sinp = nc.dram_tensor("sinp", [128, s_len], bf16, kind="ExternalInput").ap()
    m2t = nc.dram_tensor("m2t", [SQ, NKV // 4, 128, 4, 512], bf16, kind="ExternalInput").ap()
    y = nc.dram_tensor("y", [s_len, D], bf16, kind="ExternalOutput").ap()

    Act = mybir.ActivationFunctionType
    inv_sqrt_dk = 1.0 / math.sqrt(DK)

    with tile.TileContext(nc) as tc:
        with (
            tc.tile_pool(name="consts", bufs=1) as consts,
            tc.tile_pool(name="wpool", bufs=1) as wpool,
            tc.tile_pool(name="qkv", bufs=1) as qkv,
            tc.tile_pool(name="qpool", bufs=1) as qpool,
            tc.tile_pool(name="xpool", bufs=2) as xpool,
            tc.tile_pool(name="m2pool", bufs=2) as m2pool,
            tc.tile_pool(name="work1", bufs=2) as work1,
            tc.tile_pool(name="worka", bufs=2) as worka,
            tc.tile_pool(name="opool", bufs=2) as opool,
            tc.tile_pool(name="ypool", bufs=2) as ypool,
            tc.tile_pool(name="ps_proj", bufs=2, space="PSUM") as ps_proj,
            tc.tile_pool(name="ps_s", bufs=2, space="PSUM") as ps_s,
            tc.tile_pool(name="ps_o", bufs=4, space="PSUM") as ps_o,
        ):
            # ---- small constants ----
            ones128 = consts.tile([128, 128], bf16, tag="ones128", name="ones128")
            nc.vector.memset(ones128, 1.0)
            warm_rhs = consts.tile([128, 512], bf16, tag="warm", name="warm")
            nc.vector.memset(warm_rhs, 0.0)

            # ---- persistent activations (bf16) ----
            kT_sb = [[qkv.tile([128, 512], bf16, tag=f"kT_{h}_{c}", name=f"kT_{h}_{c}")
                      for c in range(SQ)] for h in range(HPC)]
            v_sb = [qkv.tile([128, DSH], bf16, tag=f"v_{i}", name=f"v_{i}")
                    for i in range(NKV)]
            wo_sb = [consts.tile([128, D], bf16, tag=f"wo_{h}", name=f"wo_{h}")
                     for h in range(HPC)]
            oT_sb = {}

            # ---------------- startup DMA schedule -----------------
            # first-needed tiles land first, fine-split across rings:
            #   scalar: wk (K proj is first), then bk/bq, then wq
            #   sync:   x chunk 0, then x prefetches / swaps / y-out
            #   vector: cos/sin, then wv
            #   gpsimd: m2 column 0, then wo, then m2 prefetches
            wq_sb = [wpool.tile([128, 4, DSH], bf16, tag=f"wq_{i}", name=f"wq_{i}")
                     for i in range(KP)]
            wk_sb = [wpool.tile([128, 4, DSH], bf16, tag=f"wk_{i}", name=f"wk_{i}")
                     for i in range(KP)]
            wv_sb = [wpool.tile([128, 4, DSH], bf16, tag=f"wv_{i}", name=f"wv_{i}")
                     for i in range(KP)]
            xcol0 = [xpool.tile([128, 4, 512], bf16, tag=f"x_{i}", name=f"x_{i}")
                     for i in range(KP)]
            for j in range(4):   # fine split so the first matmul starts early
                nc.scalar.dma_start(wk_sb[0][:, j], wkT[0, :, j])
                nc.sync.dma_start(xcol0[0][:, j], xT[0, 0, :, j])
            for i in range(1, KP):
                nc.scalar.dma_start(wk_sb[i][:], wkT[i])
                nc.sync.dma_start(xcol0[i][:], xT[0, i])
            bk_sb = consts.tile([128, HPC], f32, tag="bk", name="bk")
            nc.gpsimd.dma_start(bk_sb[:], bkp[:])
            bq_sb = consts.tile([128, HPC], f32, tag="bq", name="bq")
            nc.gpsimd.dma_start(bq_sb[:], bqp[:])
            cos_sb = consts.tile([128, s_len], bf16, tag="cos", name="cos")
            nc.gpsimd.dma_start(cos_sb[:], cosp[:])
            sin_sb = consts.tile([128, s_len], bf16, tag="sin", name="sin")
            nc.gpsimd.dma_start(sin_sb[:], sinp[:])
            # wq split across both HWDGE rings so Q(0) isn't paced by a
            # single ring still draining wk
            nc.scalar.dma_start(wq_sb[0][:], wqT[0])
            nc.sync.dma_start(wq_sb[1][:], wqT[1])
            nc.scalar.dma_start(wq_sb[2][:], wqT[2])
            nc.sync.dma_start(wq_sb[3][:], wqT[3])
            for i in range(KP):
                nc.gpsimd.dma_start(wv_sb[i][:], wvT[i])
            def m2tile(q4, i):
                # causal: ring-buffered per group-index; general masks
                # need every column resident until the attention tail.
                if causal:
                    nuse = sum(1 for qq in range(SQ)
                               if any(keep[qq][4 * i + j] for j in range(4)))
                    return m2pool.tile([128, 4, 512], bf16, tag=f"m2_{i}",
                                       bufs=min(2, nuse), name=f"m2_{i}")
                return qkv.tile([128, 4, 512], bf16, tag=f"m2_{q4}_{i}",
                                name=f"m2_{q4}_{i}")

            m2cols = {}
            m2cols[0] = [m2tile(0, i)
                         if any(keep[0][4 * i + j] for j in range(4)) else None
                         for i in range(NKV // 4)]
            for i in range(NKV // 4):
                if m2cols[0][i] is not None:
                    nc.gpsimd.dma_start(m2cols[0][i][:], m2t[0, i])
            # wo loads are emitted at the start of iteration 1 (first
            # needed by emit_y(0) mid-iteration-1), keeping the gpsimd
            # ring free for the s4=0 RoPE swaps

            # HAM warm-up: throwaway matmuls while the first DMAs land,
            # so the PE clock gate is open when real work arrives
            ps_warm = ps_s.tile([128, 512], f32, tag="ps_s", name="ps_s")
            for i in range(WARMUP_MM):
                nc.tensor.matmul(ps_warm[:], warm_rhs[:, 0:128],
                                 warm_rhs[:], start=(i == 0),
                                 stop=(i == WARMUP_MM - 1))

            # ---------------- emission helpers -----------------
            # s4==0 runs k-outer with 4 open accumulators (in the ps_o
            # pool, idle until attention) so the PE consumes each
            # 256KB contraction chunk as its DMA lands instead of
            # needing the whole 2MB weight+x before one group finishes.
            def emit_proj_qk(w_sb, b_sb, dest, s4, xcol, swap_eng):
                scol = slice(s4 * 512, (s4 + 1) * 512)
                k_outer = s4 == 0
                if k_outer:
                    pss = [ps_o.tile([128, 512], f32, tag="ps_o", name="ps_o")
                           for _ in range(HPC)]
                    for k in range(KC):
                        for mm in range(HPC):
                            nc.tensor.matmul(
                                pss[mm][:],
                                w_sb[k // 4][:, k % 4, mm * 128:(mm + 1) * 128],
                                xcol[k // 4][:, k % 4, :],
                                start=(k == 0), stop=(k == KC - 1))
                for mm in range(HPC):
                    if k_outer:
                        ps = pss[mm]
                    else:
                        ps = ps_proj.tile([128, 512], f32, tag="ps_proj",
                                          name="ps_proj")
                        for k in range(KC):
                            nc.tensor.matmul(
                                ps[:],
                                w_sb[k // 4][:, k % 4, mm * 128:(mm + 1) * 128],
                                xcol[k // 4][:, k % 4, :],
                                start=(k == 0), stop=(k == KC - 1))
                    q1 = work1.tile([128, 512], bf16, tag="q1", name="q1")
                    nc.scalar.activation(q1[:], ps[:], Act.Identity,
                                         bias=b_sb[:, mm:mm + 1])
                    # pair-swap halves via SBUF->SBUF DMA (partition
                    # shifts are not expressible on DVE/ACT lanes)
                    qsw = work1.tile([128, 512], bf16, tag="qsw", name="qsw")
                    swap_eng.dma_start(qsw[0:64], q1[64:128])
                    swap_eng.dma_start(qsw[64:128], q1[0:64])
                    tsw = work1.tile([128, 512], bf16, tag="tsw", name="tsw")
                    nc.vector.tensor_mul(tsw[:], qsw[:], sin_sb[:, scol])
                    tcs = work1.tile([128, 512], bf16, tag="tcs", name="tcs")
                    nc.vector.tensor_mul(tcs[:], q1[:], cos_sb[:, scol])
                    nc.vector.tensor_add(dest[mm][:], tcs[:], tsw[:])

            def emit_proj_v(s4, xcol):
                k_outer = s4 == 0
                if k_outer:
                    pss = [ps_o.tile([128, 512], f32, tag="ps_o", name="ps_o")
                           for _ in range(4)]
                    for k in range(KC):
                        for sl in range(4):
                            nc.tensor.matmul(
                                pss[sl][:],
                                xcol[k // 4][:, k % 4, sl * 128:(sl + 1) * 128],
                                wv_sb[k // 4][:, k % 4, :],
                                start=(k == 0), stop=(k == KC - 1))
                for sl in range(4):
                    s16 = s4 * 4 + sl
                    if k_outer:
                        ps = pss[sl]
                    else:
                        ps = ps_proj.tile([128, 512], f32, tag="ps_proj",
                                          name="ps_proj")
                        for k in range(KC):
                            nc.tensor.matmul(
                                ps[:],
                                xcol[k // 4][:, k % 4, sl * 128:(sl + 1) * 128],
                                wv_sb[k // 4][:, k % 4, :],
                                start=(k == 0), stop=(k == KC - 1))
                    nc.vector.tensor_copy(v_sb[s16][:], ps[:])

            attn_st = {}

            def attn_blocks(h, q4, qT, kvs, m2col, first, last):
                """Score/exp/mask/AV chain for a slice of q-col q4's kept
                kv blocks; `first` opens the PSUM chains, `last` closes
                them and emits the normalization."""
                if first:
                    attn_st[(h, q4)] = st = {
                        "ps_oT": ps_o.tile([128, 512], f32, tag="ps_o",
                                           name="ps_o"),
                        "gacc": worka.tile([128, 512], bf16,
                                           tag=f"gacc{h % 2}", bufs=2,
                                           name="gacc"),
                        "idx": 0,
                    }
                else:
                    st = attn_st[(h, q4)]
                ps_oT, gacc = st["ps_oT"], st["gacc"]
                nkept = sum(1 for kv in range(NKV) if keep[q4][kv])
                for kv in kvs:
                    idx = st["idx"]
                    sc = scols[q4][kv]
                    ps_sc = ps_s.tile([128, 512], f32, tag="ps_s", name="ps_s")
                    nc.tensor.matmul(
                        ps_sc[:, sc:],
                        kT_sb[h][kv // 4][:, (kv % 4) * 128:(kv % 4 + 1) * 128],
                        qT[:, sc:], start=True, stop=True)
                    e = worka.tile([128, 512], bf16, tag="e", bufs=3,
                                   name="e")
                    nc.scalar.activation(e[:, sc:], ps_sc[:, sc:], Act.Exp,
                                         scale=inv_sqrt_dk)
                    g = worka.tile([128, 512], bf16, tag=f"g{idx % 2}",
                                   bufs=2, name="g")
                    nc.vector.tensor_mul(g[:, sc:], e[:, sc:],
                                         m2col[kv // 4][:, kv % 4, sc:])
                    nc.tensor.matmul(
                        ps_oT[:, sc:], v_sb[kv][:, h * 128:(h + 1) * 128],
                        g[:, sc:], start=(idx == 0), stop=(idx == nkept - 1))
                    if idx == 0:
                        assert sc == 0
                        nc.vector.tensor_copy(gacc[:], g[:])
                    else:
                        nc.vector.tensor_add(gacc[:, sc:], gacc[:, sc:],
                                             g[:, sc:])
                    st["idx"] = idx + 1
                if last:
                    # denominator, broadcast to all partitions by an
                    # all-ones stationary operand; then one full-width
                    # fast-reciprocal (uses all 128 DVE lanes)
                    ps_D = ps_s.tile([128, 512], f32, tag="ps_s", name="ps_s")
                    nc.tensor.matmul(ps_D[:], ones128[:], gacc[:],
                                     start=True, stop=True)
                    rcp = worka.tile([128, 512], f32, tag="rcp", bufs=1,
                                     name="rcp")
                    nc.vector.reciprocal_approx_fast(rcp[:], ps_D[:])
                    nc.vector.tensor_mul(oT_sb[q4][h][:], ps_oT[:], rcp[:])

            def emit_y(q4):
                final = q4 == SQ - 1
                rings = [nc.sync, nc.scalar, nc.gpsimd]
                for sl in range(4):
                    srow = slice((q4 * 4 + sl) * 128, (q4 * 4 + sl + 1) * 128)
                    lrow = slice(sl * 128, (sl + 1) * 128)
                    ysb = ypool.tile([128, D], bf16, tag="ysb", name="ysb")
                    for j4 in range(JQ):
                        jcol = slice(j4 * 512, (j4 + 1) * 512)
                        ps_y = ps_proj.tile([128, 512], f32, tag="ps_proj",
                                            name="ps_proj")
                        for h in range(HPC):
                            nc.tensor.matmul(
                                ps_y[:], oT_sb[q4][h][:, lrow], wo_sb[h][:, jcol],
                                start=(h == 0), stop=(h == HPC - 1))
                        if j4 % 2 == 0:
                            nc.scalar.copy(ysb[:, jcol], ps_y[:])
                        else:
                            nc.vector.tensor_copy(ysb[:, jcol], ps_y[:])
                        if final:
                            # nothing left to hide behind: ship each piece
                            # as soon as its evacuation lands, round-robin
                            # over the three DMA-capable rings
                            rings[(sl * JQ + j4) % 3].dma_start(
                                y[srow, jcol], ysb[:, jcol])
                    if not final:
                        # scalar ring: free after the startup weight loads
                        nc.scalar.dma_start(y[srow, :], ysb[:])

            # ================= main pipelined loop =================
            xcols = {0: xcol0}
            qT_keep = {}

            def emit_x_prefetch(s4):
                # late in the iteration so this iteration's RoPE swaps
                # get the sync ring first; still a full y-column of
                # lead time before the next projections need it
                if s4 + 1 < SQ:
                    xcols[s4 + 1] = [xpool.tile([128, 4, 512], bf16,
                                                tag=f"x_{i}", name=f"x_{i}")
                                     for i in range(KP)]
                    for i in range(KP):
                        nc.sync.dma_start(xcols[s4 + 1][i][:], xT[s4 + 1, i])

            for s4 in range(SQ):
                # prefetch next m2 column early (small, gpsimd ring)
                if s4 + 1 < SQ:
                    m2cols[s4 + 1] = [
                        m2tile(s4 + 1, i)
                        if any(keep[s4 + 1][4 * i + j] for j in range(4))
                        else None for i in range(NKV // 4)]
                    for i in range(NKV // 4):
                        if m2cols[s4 + 1][i] is not None:
                            nc.gpsimd.dma_start(m2cols[s4 + 1][i][:],
                                                m2t[s4 + 1, i])
                if s4 == 1:
                    for h in range(HPC):
                        nc.gpsimd.dma_start(wo_sb[h][:],
                                            woT[h * 128:(h + 1) * 128, :])

                swap_eng = nc.sync
                xcol = xcols[s4]
                emit_proj_qk(wk_sb, bk_sb, [kT_sb[h][s4] for h in range(HPC)],
                             s4, xcol, swap_eng)
                if causal:
                    qT_cur = [qpool.tile([128, 512], bf16, tag=f"qT_{h}",
                                         name=f"qT_{h}") for h in range(HPC)]
                else:
                    qT_cur = [qkv.tile([128, 512], bf16, tag=f"qT_{h}_{s4}",
                                       name=f"qT_{h}_{s4}")
                              for h in range(HPC)]
                emit_proj_qk(wq_sb, bq_sb, qT_cur, s4, xcol, swap_eng)

                if causal:
                    oT_sb[s4] = [opool.tile([128, 512], bf16, tag=f"oT_{h}",
                                            name=f"oT_{h}")
                                 for h in range(HPC)]
                    nd = [kv for kv in range(NKV)
                          if keep[s4][kv] and kv // 4 < s4]
                    dg = [kv for kv in range(NKV)
                          if keep[s4][kv] and kv // 4 == s4]
                    assert nd or dg, "fully masked query column"
                    for h in range(HPC):
                        if nd:
                            attn_blocks(h, s4, qT_cur[h], nd, m2cols[s4],
                                        first=True, last=False)
                    emit_proj_v(s4, xcol)
                    for h in range(HPC):
                        attn_blocks(h, s4, qT_cur[h], dg, m2cols[s4],
                                    first=not nd, last=True)
                    emit_x_prefetch(s4)
                    if s4 > 0:
                        emit_y(s4 - 1)
                else:
                    emit_proj_v(s4, xcol)
                    emit_x_prefetch(s4)
                    qT_keep[s4] = qT_cur

            if causal:
                emit_y(SQ - 1)
            else:
                # general mask: attention needs arbitrary kv chunks, so
                # it all runs after the projections (qT kept per chunk)
                for q4 in range(SQ):
                    oT_sb[q4] = [opool.tile([128, 512], bf16, tag=f"oT_{h}",
                                            name=f"oT_{h}")
                                 for h in range(HPC)]
                    kept = [kv for kv in range(NKV) if keep[q4][kv]]
                    assert kept, "fully masked query column"
                    for h in range(HPC):
                        attn_blocks(h, q4, qT_keep[q4][h], kept, m2cols[q4],
                                    first=True, last=True)
                    if q4 > 0:
                        emit_y(q4 - 1)
                emit_y(SQ - 1)

    nc.compile()
    return nc


def _rope_perm():
    """Within each head's 128 rows: evens first, then odds."""
    base = np.concatenate([np.arange(0, 128, 2), np.arange(1, 128, 2)])
    return np.concatenate([h * 128 + base for h in range(HPC)])


def _blk(a):
    """[R, C] -> [C//512, R//512, 128, 4, 512] packed contiguous blocks.

    Block [c4, i, :, j, :] = a[(4*i+j)*128:(4*i+j+1)*128, c4*512:(c4+1)*512].
    """
    r, c = a.shape
    return np.ascontiguousarray(
        a.reshape(r // 512, 4, 128, c // 512, 512).transpose(3, 0, 2, 1, 4))


def _wpack(a):
    """[R, C] -> [R//512, 128, 4, C]: pack 4 row-chunks per tile."""
    r, c = a.shape
    return np.ascontiguousarray(
        a.reshape(r // 512, 4, 128, c).transpose(0, 2, 1, 3))


def prepare_inputs(x, freqs, hard_mask, soft_mask, wq, bq, wk, bk, wv, wo,
                   s_len=S):
    """Host-side shard + layout prep.  Returns one in_map per core."""
    perm = _rope_perm()
    cos = np.cos(np.asarray(freqs, np.float32))   # [S, 64]
    sin = np.sin(np.asarray(freqs, np.float32))
    cosp = np.ascontiguousarray(
        np.concatenate([cos.T, cos.T], axis=0)).astype(BF16)     # [128, S]
    sinp = np.ascontiguousarray(
        np.concatenate([-sin.T, sin.T], axis=0)).astype(BF16)
    hard = np.asarray(hard_mask, np.float32).reshape(s_len, s_len)
    soft = np.asarray(soft_mask, np.float32).reshape(s_len, s_len)
    m2t = _blk((hard * (soft + 1e-6)).T.astype(BF16))

    xT = [_blk(np.asarray(x[b], np.float32).T.astype(BF16)) for b in range(B)]

    per_group = []
    for hg in range(4):
        rows = slice(DSH * hg, DSH * (hg + 1))
        wq_sh = np.asarray(wq, np.float32)[rows][perm]
        wk_sh = np.asarray(wk, np.float32)[rows][perm]
        wv_sh = np.asarray(wv, np.float32)[rows]
        per_group.append({
            "wqT": _wpack(np.ascontiguousarray(wq_sh.T).astype(BF16)),
            "wkT": _wpack(np.ascontiguousarray(wk_sh.T).astype(BF16)),
            "wvT": _wpack(np.ascontiguousarray(wv_sh.T).astype(BF16)),
            "woT": np.ascontiguousarray(
                np.asarray(wo, np.float32)[:, rows].T).astype(BF16),
            "bqp": np.ascontiguousarray(
                np.asarray(bq, np.float32)[rows][perm].reshape(HPC, 128).T),
            "bkp": np.ascontiguousarray(
                np.asarray(bk, np.float32)[rows][perm].reshape(HPC, 128).T),
        })

    in_maps = []
    for core in range(N_CORES):
        b, hg = core // 4, core % 4
        m = {"xT": xT[b], "cosp": cosp, "sinp": sinp, "m2t": m2t}
        m.update(per_group[hg])
        in_maps.append(m)
    return in_maps


def _mask_pattern(m2b):
    """Block keep flags + per-block leading-zero column offsets."""
    SQ_, NG = m2b.shape[0], m2b.shape[1]
    keep, scols = [], []
    for q4 in range(SQ_):
        krow, srow = [], []
        for kv in range(NG * 4):
            blk = m2b[q4, kv // 4, :, kv % 4]            # [128, 512]
            cols = np.nonzero(np.any(blk != 0, axis=0))[0]
            if cols.size == 0:
                krow.append(False)
                srow.append(0)
            else:
                krow.append(True)
                srow.append(int(cols[0]) // 128 * 128)
        first = next((kv for kv in range(NG * 4) if krow[kv]), None)
        if first is not None:
            srow[first] = 0   # chain head initializes full PSUM width
        keep.append(tuple(krow))
        scols.append(tuple(srow))
    return tuple(keep), tuple(scols)


def kernel(x, freqs, hard_mask, soft_mask, wq, bq, wk, bk, wv, bv, wo, bo,
           _trace=False, _tmpdir=None):
    s_len = x.shape[1]
    in_maps = prepare_inputs(x, freqs, hard_mask, soft_mask, wq, bq, wk, bk,
                             wv, wo, s_len=s_len)
    keep, scols = _mask_pattern(in_maps[0]["m2t"])
    ckey = (s_len, keep, scols)
    if ckey not in _NC_CACHE:
        _NC_CACHE[ckey] = build_bass(s_len, keep, scols)
    nc = _NC_CACHE[ckey]
    kwargs = {}
    if _trace:
        kwargs = {"trace": True, "tmpdir": _tmpdir}
    res = run_bass_kernel_spmd(nc, in_maps, core_ids=list(range(N_CORES)),
                               **kwargs)
    # bv folded out of the kernel: sum_kv attn == 1, so + bv per channel
    # post-softmax, and through wo that is the constant wo @ bv.
    bo_eff = (np.asarray(bo, np.float32)
              + np.asarray(wo, np.float32) @ np.asarray(bv, np.float32))
    out = np.empty((B, s_len, D), np.float32)
    for b in range(B):
        acc = res.results[4 * b]["y"].astype(np.float32)
        for hg in range(1, 4):
            acc = acc + res.results[4 * b + hg]["y"].astype(np.float32)
        out[b] = acc + bo_eff[None, :]
    kernel.last_result = res
    return out


# revision 29
# speedup vs baseline: 1.1607x; 1.1607x over previous
"""Trainium2 Bass kernel for nn_Attention_39049842655427.

Multi-head attention (RoPE + hard mask + soft gate mask) over 8
NeuronCores: data-parallel over batch (2) x tensor-parallel over heads
(16 heads -> 4 per core).  Each core computes q/k/v projections for its
4 heads, the head-sharded attention, and a partial output projection
(wo row-sharded); the host sums the 4 partials per batch and adds bo.

Math notes (exact up to float rounding):
  reference:  e = exp(s)*hard ; a1 = e/sum(e) ; a2 = a1*soft
              attn = a2/(sum(a2)+1e-6) ; out = attn @ v
  identity:   attn = f / (F + 1e-6*E),  f = e*hard*soft,
              F = sum(f), E = sum(e*hard)
  kernel:     g = exp(s) * M2,  M2 = hard*(soft+1e-6)
              => sum(g) = F + 1e-6*E exactly; numerator uses g instead
              of f, an O(1e-6) perturbation of attn.
  bv folding: sum_kv attn = 1 exactly under the g/sum(g) form, so the
              v-bias contributes bv per output channel post-softmax;
              it is moved to the host as bo_eff = bo + wo @ bv.
All matmuls run in bf16 with fp32 PSUM accumulation.  Scores are
computed transposed (s[kv,q]) so attn@v needs no on-device transpose;
RoPE pairs are pre-permuted (evens then odds) inside each head's 128
rows of wq/wk so the rotation partner is a partition offset of 64
(applied via a small SBUF->SBUF DMA).  128x512 blocks of M2 that are
exactly zero are skipped entirely, and within kept blocks leading
all-zero column spans (in 128 steps) narrow the score/exp/mask/AV work
(exact, data-adaptive: read from the actual mask and baked into the
compiled program; a dense mask falls back to the all-keep program).

For a causal keep pattern the whole kernel is a single software-
pipelined loop over 512-row chunks: K/Q projections for chunk s4, then
the off-diagonal attention blocks of query-column s4 (their k/v chunks
are older), then the V projection of chunk s4, then the diagonal
attention blocks and the softmax normalization, then the output
projection of column s4-1.  This keeps the scalar engine's exp work
spread across the projection matmuls instead of piling up in a
separate attention phase.
"""

import math
import sys

import numpy as np
import ml_dtypes

if "/opt/trn_rl_repo" not in sys.path:
    sys.path.append("/opt/trn_rl_repo")

import concourse.bass as bass  # noqa: E402,F401
import concourse.tile as tile  # noqa: E402
from concourse import bacc, mybir  # noqa: E402
from concourse.bass_utils import run_bass_kernel_spmd  # noqa: E402

B, S, D, H, DK = 2, 2048, 2048, 16, 128
N_CORES = 8
HPC = 4          # heads per core
DSH = HPC * DK   # 512, d-shard per core

BF16 = ml_dtypes.bfloat16

_NC_CACHE = {}

WARMUP_MM = 14   # HAM warm-up matmuls bridging the startup DMA window


def build_bass(s_len=S, keep=None, scols=None):
    """Build the SPMD single-core program (same NEFF on all 8 cores)."""
    f32 = mybir.dt.float32
    bf16 = mybir.dt.bfloat16
    KC = D // 128          # contraction chunks for projections
    SQ = s_len // 512      # 512-wide q/s chunks
    NKV = s_len // 128     # 128-row kv chunks
    JQ = D // 512          # output-column chunks
    KP = KC // 4
    if keep is None:
        keep = tuple(tuple(True for _ in range(NKV)) for _ in range(SQ))
    if scols is None:
        scols = tuple(tuple(0 for _ in range(NKV)) for _ in range(SQ))
    # causal pattern: every kept kv block of column q4 lives in chunks
    # <= q4, which is what lets attention interleave into the
    # projection loop.
    causal = all(kv // 4 <= q4
                 for q4 in range(SQ) for kv in range(NKV) if keep[q4][kv])

    nc = bacc.Bacc("TRN2", target_bir_lowering=False, debug=False,
                   num_devices=N_CORES)

    xT = nc.dram_tensor("xT", [SQ, KP, 128, 4, 512], bf16, kind="ExternalInput").ap()
    wqT = nc.dram_tensor("wqT", [KP, 128, 4, DSH], bf16, kind="ExternalInput").ap()
    wkT = nc.dram_tensor("wkT", [KP, 128, 4, DSH], bf16, kind="ExternalInput").ap()
    wvT = nc.dram_tensor("wvT", [KP, 128, 4, DSH], bf16, kind="ExternalInput").ap()
    woT = nc.dram_tensor("woT", [DSH, D], bf16, kind="ExternalInput").ap()
    bqp = nc.dram_tensor("bqp", [128, HPC], f32, kind="ExternalInput").ap()
    bkp = nc.dram_tensor("bkp", [128, HPC], f32, kind="ExternalInput").ap()
    cosp = nc.dram_tensor("cosp", [128, s_len], bf16, kind="ExternalInput").ap()
    sinp = nc.dram_tensor("sinp", [128, s_len], bf16, kind="ExternalInput").ap()
    m2t = nc.dram_tensor("m2t", [SQ, NKV // 4, 128, 4, 512], bf16, kind="ExternalInput").ap()
    y = nc.dram_tensor("y", [s_len, D], bf16, kind="ExternalOutput").ap()

    Act = mybir.ActivationFunctionType
    inv_sqrt_dk = 1.0 / math.sqrt(DK)

    with tile.TileContext(nc) as tc:
        with (
            tc.tile_pool(name="consts", bufs=1) as consts,
            tc.tile_pool(name="wpool", bufs=1) as wpool,
            tc.tile_pool(name="qkv", bufs=1) as qkv,
            tc.tile_pool(name="qpool", bufs=1) as qpool,
            tc.tile_pool(name="xpool", bufs=2) as xpool,
            tc.tile_pool(name="m2pool", bufs=2) as m2pool,
            tc.tile_pool(name="work1", bufs=2) as work1,
            tc.tile_pool(name="worka", bufs=2) as worka,
            tc.tile_pool(name="opool", bufs=2) as opool,
            tc.tile_pool(name="ypool", bufs=2) as ypool,
            tc.tile_pool(name="ps_proj", bufs=2, space="PSUM") as ps_proj,
            tc.tile_pool(name="ps_s", bufs=2, space="PSUM") as ps_s,
            tc.tile_pool(name="ps_o", bufs=4, space="PSUM") as ps_o,
        ):
            # ---- small constants ----
            ones128 = consts.tile([128, 128], bf16, tag="ones128", name="ones128")
            nc.vector.memset(ones128, 1.0)
            warm_rhs = consts.tile([128, 512], bf16, tag="warm", name="warm")
            nc.vector.memset(warm_rhs, 0.0)

            # ---- persistent activations (bf16) ----
            kT_sb = [[qkv.tile([128, 512], bf16, tag=f"kT_{h}_{c}", name=f"kT_{h}_{c}")
                      for c in range(SQ)] for h in range(HPC)]
            v_sb = [qkv.tile([128, DSH], bf16, tag=f"v_{i}", name=f"v_{i}")
                    for i in range(NKV)]
            wo_sb = [consts.tile([128, D], bf16, tag=f"wo_{h}", name=f"wo_{h}")
                     for h in range(HPC)]
            oT_sb = {}

            # ---------------- startup DMA schedule -----------------
            # first-needed tiles land first, fine-split across rings:
            #   scalar: wk (K proj is first), then bk/bq, then wq
            #   sync:   x chunk 0, then x prefetches / swaps / y-out
            #   vector: cos/sin, then wv
            #   gpsimd: m2 column 0, then wo, then m2 prefetches
            wq_sb = [wpool.tile([128, 4, DSH], bf16, tag=f"wq_{i}", name=f"wq_{i}")
                     for i in range(KP)]
            wk_sb = [wpool.tile([128, 4, DSH], bf16, tag=f"wk_{i}", name=f"wk_{i}")
                     for i in range(KP)]
            wv_sb = [wpool.tile([128, 4, DSH], bf16, tag=f"wv_{i}", name=f"wv_{i}")
                     for i in range(KP)]
            xcol0 = [xpool.tile([128, 4, 512], bf16, tag=f"x_{i}", name=f"x_{i}")
                     for i in range(KP)]
            for j in range(4):   # fine split so the first matmul starts early
                nc.scalar.dma_start(wk_sb[0][:, j], wkT[0, :, j])
                nc.sync.dma_start(xcol0[0][:, j], xT[0, 0, :, j])
            for i in range(1, KP):
                nc.scalar.dma_start(wk_sb[i][:], wkT[i])
                nc.sync.dma_start(xcol0[i][:], xT[0, i])
            bk_sb = consts.tile([128, HPC], f32, tag="bk", name="bk")
            nc.gpsimd.dma_start(bk_sb[:], bkp[:])
            bq_sb = consts.tile([128, HPC], f32, tag="bq", name="bq")
            nc.gpsimd.dma_start(bq_sb[:], bqp[:])
            cos_sb = consts.tile([128, s_len], bf16, tag="cos", name="cos")
            nc.gpsimd.dma_start(cos_sb[:], cosp[:])
            sin_sb = consts.tile([128, s_len], bf16, tag="sin", name="sin")
            nc.gpsimd.dma_start(sin_sb[:], sinp[:])
            # wq split across both HWDGE rings so Q(0) isn't paced by a
            # single ring still draining wk
            nc.scalar.dma_start(wq_sb[0][:], wqT[0])
            nc.sync.dma_start(wq_sb[1][:], wqT[1])
            nc.scalar.dma_start(wq_sb[2][:], wqT[2])
            nc.sync.dma_start(wq_sb[3][:], wqT[3])
            for i in range(KP):
                nc.gpsimd.dma_start(wv_sb[i][:], wvT[i])
            def m2tile(q4, i):
                # causal: ring-buffered per group-index; general masks
                # need every column resident until the attention tail.
                if causal:
                    nuse = sum(1 for qq in range(SQ)
                               if any(keep[qq][4 * i + j] for j in range(4)))
                    return m2pool.tile([128, 4, 512], bf16, tag=f"m2_{i}",
                                       bufs=min(2, nuse), name=f"m2_{i}")
                return qkv.tile([128, 4, 512], bf16, tag=f"m2_{q4}_{i}",
                                name=f"m2_{q4}_{i}")

            m2cols = {}
            m2cols[0] = [m2tile(0, i)
                         if any(keep[0][4 * i + j] for j in range(4)) else None
                         for i in range(NKV // 4)]
            for i in range(NKV // 4):
                if m2cols[0][i] is not None:
                    nc.gpsimd.dma_start(m2cols[0][i][:], m2t[0, i])
            # wo loads are emitted at the start of iteration 1 (first
            # needed by emit_y(0) mid-iteration-1), keeping the gpsimd
            # ring free for the s4=0 RoPE swaps

            # HAM warm-up: throwaway matmuls while the first DMAs land,
            # so the PE clock gate is open when real work arrives
            ps_warm = ps_s.tile([128, 512], f32, tag="ps_s", name="ps_s")
            for i in range(WARMUP_MM):
                nc.tensor.matmul(ps_warm[:], warm_rhs[:, 0:128],
                                 warm_rhs[:], start=(i == 0),
                                 stop=(i == WARMUP_MM - 1))

            # ---------------- emission helpers -----------------
            # s4==0 runs k-outer with 4 open accumulators (in the ps_o
            # pool, idle until attention) so the PE consumes each
            # 256KB contraction chunk as its DMA lands instead of
            # needing the whole 2MB weight+x before one group finishes.
            def emit_proj_qk(w_sb, b_sb, dest, s4, xcol, swap_eng):
                scol = slice(s4 * 512, (s4 + 1) * 512)
                k_outer = s4 == 0
                if k_outer:
                    pss = [ps_o.tile([128, 512], f32, tag="ps_o", name="ps_o")
                           for _ in range(HPC)]
                    for k in range(KC):
                        for mm in range(HPC):
                            nc.tensor.matmul(
                                pss[mm][:],
                                w_sb[k // 4][:, k % 4, mm * 128:(mm + 1) * 128],
                                xcol[k // 4][:, k % 4, :],
                                start=(k == 0), stop=(k == KC - 1))
                for mm in range(HPC):
                    if k_outer:
                        ps = pss[mm]
                    else:
                        ps = ps_proj.tile([128, 512], f32, tag="ps_proj",
                                          name="ps_proj")
                        for k in range(KC):
                            nc.tensor.matmul(
                                ps[:],
                                w_sb[k // 4][:, k % 4, mm * 128:(mm + 1) * 128],
                                xcol[k // 4][:, k % 4, :],
                                start=(k == 0), stop=(k == KC - 1))
                    q1 = work1.tile([128, 512], bf16, tag="q1", name="q1")
                    nc.scalar.activation(q1[:], ps[:], Act.Identity,
                                         bias=b_sb[:, mm:mm + 1])
                    # pair-swap halves via SBUF->SBUF DMA (partition
                    # shifts are not expressible on DVE/ACT lanes)
                    qsw = work1.tile([128, 512], bf16, tag="qsw", name="qsw")
                    swap_eng.dma_start(qsw[0:64], q1[64:128])
                    swap_eng.dma_start(qsw[64:128], q1[0:64])
                    tsw = work1.tile([128, 512], bf16, tag="tsw", name="tsw")
                    nc.vector.tensor_mul(tsw[:], qsw[:], sin_sb[:, scol])
                    tcs = work1.tile([128, 512], bf16, tag="tcs", name="tcs")
                    nc.vector.tensor_mul(tcs[:], q1[:], cos_sb[:, scol])
                    nc.vector.tensor_add(dest[mm][:], tcs[:], tsw[:])

            def emit_proj_v(s4, xcol):
                k_outer = s4 == 0
                if k_outer:
                    pss = [ps_o.tile([128, 512], f32, tag="ps_o", name="ps_o")
                           for _ in range(4)]
                    for k in range(KC):
                        for sl in range(4):
                            nc.tensor.matmul(
                                pss[sl][:],
                                xcol[k // 4][:, k % 4, sl * 128:(sl + 1) * 128],
                                wv_sb[k // 4][:, k % 4, :],
                                start=(k == 0), stop=(k == KC - 1))
                for sl in range(4):
                    s16 = s4 * 4 + sl
                    if k_outer:
                        ps = pss[sl]
                    else:
                        ps = ps_proj.tile([128, 512], f32, tag="ps_proj",
                                          name="ps_proj")
                        for k in range(KC):
                            nc.tensor.matmul(
                                ps[:],
                                xcol[k // 4][:, k % 4, sl * 128:(sl + 1) * 128],
                                wv_sb[k // 4][:, k % 4, :],
                                start=(k == 0), stop=(k == KC - 1))
                    nc.vector.tensor_copy(v_sb[s16][:], ps[:])

            attn_st = {}

            def attn_blocks(h, q4, qT, kvs, m2col, first, last):
                """Score/exp/mask/AV chain for a slice of q-col q4's kept
                kv blocks; `first` opens the PSUM chains, `last` closes
                them and emits the normalization."""
                if first:
                    attn_st[(h, q4)] = st = {
                        "ps_oT": ps_o.tile([128, 512], f32, tag="ps_o",
                                           name="ps_o"),
                        "gacc": worka.tile([128, 512], bf16,
                                           tag=f"gacc{h % 2}", bufs=2,
                                           name="gacc"),
                        "idx": 0,
                    }
                else:
                    st = attn_st[(h, q4)]
                ps_oT, gacc = st["ps_oT"], st["gacc"]
                nkept = sum(1 for kv in range(NKV) if keep[q4][kv])
                for kv in kvs:
                    idx = st["idx"]
                    sc = scols[q4][kv]
                    ps_sc = ps_s.tile([128, 512], f32, tag="ps_s", name="ps_s")
                    nc.tensor.matmul(
                        ps_sc[:, sc:],
                        kT_sb[h][kv // 4][:, (kv % 4) * 128:(kv % 4 + 1) * 128],
                        qT[:, sc:], start=True, stop=True)
                    e = worka.tile([128, 512], bf16, tag="e", bufs=3,
                                   name="e")
                    nc.scalar.activation(e[:, sc:], ps_sc[:, sc:], Act.Exp,
                                         scale=inv_sqrt_dk)
                    g = worka.tile([128, 512], bf16, tag=f"g{idx % 2}",
                                   bufs=2, name="g")
                    nc.vector.tensor_mul(g[:, sc:], e[:, sc:],
                                         m2col[kv // 4][:, kv % 4, sc:])
                    nc.tensor.matmul(
                        ps_oT[:, sc:], v_sb[kv][:, h * 128:(h + 1) * 128],
                        g[:, sc:], start=(idx == 0), stop=(idx == nkept - 1))
                    if idx == 0:
                        assert sc == 0
                        nc.vector.tensor_copy(gacc[:], g[:])
                    else:
                        nc.vector.tensor_add(gacc[:, sc:], gacc[:, sc:],
                                             g[:, sc:])
                    st["idx"] = idx + 1
                if last:
                    # denominator, broadcast to all partitions by an
                    # all-ones stationary operand; then one full-width
                    # fast-reciprocal (uses all 128 DVE lanes)
                    ps_D = ps_s.tile([128, 512], f32, tag="ps_s", name="ps_s")
                    nc.tensor.matmul(ps_D[:], ones128[:], gacc[:],
                                     start=True, stop=True)
                    rcp = worka.tile([128, 512], f32, tag="rcp", bufs=1,
                                     name="rcp")
                    nc.vector.reciprocal_approx_fast(rcp[:], ps_D[:])
                    nc.vector.tensor_mul(oT_sb[q4][h][:], ps_oT[:], rcp[:])

            def emit_y(q4):
                final = q4 == SQ - 1
                rings = [nc.sync, nc.scalar, nc.gpsimd]
                for sl in range(4):
                    srow = slice((q4 * 4 + sl) * 128, (q4 * 4 + sl + 1) * 128)
                    lrow = slice(sl * 128, (sl + 1) * 128)
                    ysb = ypool.tile([128, D], bf16, tag="ysb", name="ysb")
                    for j4 in range(JQ):
                        jcol = slice(j4 * 512, (j4 + 1) * 512)
                        ps_y = ps_proj.tile([128, 512], f32, tag="ps_proj",
                                            name="ps_proj")
                        for h in range(HPC):
                            nc.tensor.matmul(
                                ps_y[:], oT_sb[q4][h][:, lrow], wo_sb[h][:, jcol],
                                start=(h == 0), stop=(h == HPC - 1))
                        if j4 % 2 == 0:
                            nc.scalar.copy(ysb[:, jcol], ps_y[:])
                        else:
                            nc.vector.tensor_copy(ysb[:, jcol], ps_y[:])
                        if final:
                            # nothing left to hide behind: ship each piece
                            # as soon as its evacuation lands, round-robin
                            # over the three DMA-capable rings
                            rings[(sl * JQ + j4) % 3].dma_start(
                                y[srow, jcol], ysb[:, jcol])
                    if not final:
                        # scalar ring: free after the startup weight loads
                        nc.scalar.dma_start(y[srow, :], ysb[:])

            # ================= main pipelined loop =================
            xcols = {0: xcol0}
            qT_keep = {}

            def emit_x_prefetch(s4):
                # mid-iteration on the scalar ring (which only carries
                # y output rows, tolerant of delay) so the next chunk is
                # resident before the next iteration's projections
                if s4 + 1 < SQ:
                    xcols[s4 + 1] = [xpool.tile([128, 4, 512], bf16,
                                                tag=f"x_{i}", name=f"x_{i}")
                                     for i in range(KP)]
                    for i in range(KP):
                        nc.scalar.dma_start(xcols[s4 + 1][i][:], xT[s4 + 1, i])

            for s4 in range(SQ):
                # prefetch next m2 column early (small, gpsimd ring)
                if s4 + 1 < SQ:
                    m2cols[s4 + 1] = [
                        m2tile(s4 + 1, i)
                        if any(keep[s4 + 1][4 * i + j] for j in range(4))
                        else None for i in range(NKV // 4)]
                    for i in range(NKV // 4):
                        if m2cols[s4 + 1][i] is not None:
                            nc.gpsimd.dma_start(m2cols[s4 + 1][i][:],
                                                m2t[s4 + 1, i])
                if s4 == 1:
                    for h in range(HPC):
                        nc.gpsimd.dma_start(wo_sb[h][:],
                                            woT[h * 128:(h + 1) * 128, :])

                swap_eng = nc.sync
                xcol = xcols[s4]
                emit_proj_qk(wk_sb, bk_sb, [kT_sb[h][s4] for h in range(HPC)],
                             s4, xcol, swap_eng)
                if causal:
                    qT_cur = [qpool.tile([128, 512], bf16, tag=f"qT_{h}",
                                         name=f"qT_{h}") for h in range(HPC)]
                else:
                    qT_cur = [qkv.tile([128, 512], bf16, tag=f"qT_{h}_{s4}",
                                       name=f"qT_{h}_{s4}")
                              for h in range(HPC)]
                emit_proj_qk(wq_sb, bq_sb, qT_cur, s4, xcol, swap_eng)

                if causal:
                    oT_sb[s4] = [opool.tile([128, 512], bf16, tag=f"oT_{h}",
                                            name=f"oT_{h}")
                                 for h in range(HPC)]
                    nd = [kv for kv in range(NKV)
                          if keep[s4][kv] and kv // 4 < s4]
                    dg = [kv for kv in range(NKV)
                          if keep[s4][kv] and kv // 4 == s4]
                    assert nd or dg, "fully masked query column"
                    for h in range(HPC):
                        if nd:
                            attn_blocks(h, s4, qT_cur[h], nd, m2cols[s4],
                                        first=True, last=False)
                    emit_x_prefetch(s4)
                    emit_proj_v(s4, xcol)
                    for h in range(HPC):
                        attn_blocks(h, s4, qT_cur[h], dg, m2cols[s4],
                                    first=not nd, last=True)
                    if s4 > 0:
                        emit_y(s4 - 1)
                else:
                    emit_proj_v(s4, xcol)
                    emit_x_prefetch(s4)
                    qT_keep[s4] = qT_cur

            if causal:
                emit_y(SQ - 1)
            else:
                # general mask: attention needs arbitrary kv chunks, so
                # it all runs after the projections (qT kept per chunk)
                for q4 in range(SQ):
                    oT_sb[q4] = [opool.tile([128, 512], bf16, tag=f"oT_{h}",
                                            name=f"oT_{h}")
                                 for h in range(HPC)]
                    kept = [kv for kv in range(NKV) if keep[q4][kv]]
                    assert kept, "fully masked query column"
                    for h in range(HPC):
                        attn_blocks(h, q4, qT_keep[q4][h], kept, m2cols[q4],
                                    first=True, last=True)
                    if q4 > 0:
                        emit_y(q4 - 1)
                emit_y(SQ - 1)

    nc.compile()
    return nc


def _rope_perm():
    """Within each head's 128 rows: evens first, then odds."""
    base = np.concatenate([np.arange(0, 128, 2), np.arange(1, 128, 2)])
    return np.concatenate([h * 128 + base for h in range(HPC)])


def _blk(a):
    """[R, C] -> [C//512, R//512, 128, 4, 512] packed contiguous blocks.

    Block [c4, i, :, j, :] = a[(4*i+j)*128:(4*i+j+1)*128, c4*512:(c4+1)*512].
    """
    r, c = a.shape
    return np.ascontiguousarray(
        a.reshape(r // 512, 4, 128, c // 512, 512).transpose(3, 0, 2, 1, 4))


def _wpack(a):
    """[R, C] -> [R//512, 128, 4, C]: pack 4 row-chunks per tile."""
    r, c = a.shape
    return np.ascontiguousarray(
        a.reshape(r // 512, 4, 128, c).transpose(0, 2, 1, 3))


def prepare_inputs(x, freqs, hard_mask, soft_mask, wq, bq, wk, bk, wv, wo,
                   s_len=S):
    """Host-side shard + layout prep.  Returns one in_map per core."""
    perm = _rope_perm()
    cos = np.cos(np.asarray(freqs, np.float32))   # [S, 64]
    sin = np.sin(np.asarray(freqs, np.float32))
    cosp = np.ascontiguousarray(
        np.concatenate([cos.T, cos.T], axis=0)).astype(BF16)     # [128, S]
    sinp = np.ascontiguousarray(
        np.concatenate([-sin.T, sin.T], axis=0)).astype(BF16)
    hard = np.asarray(hard_mask, np.float32).reshape(s_len, s_len)
    soft = np.asarray(soft_mask, np.float32).reshape(s_len, s_len)
    m2t = _blk((hard * (soft + 1e-6)).T.astype(BF16))

    xT = [_blk(np.asarray(x[b], np.float32).T.astype(BF16)) for b in range(B)]

    per_group = []
    for hg in range(4):
        rows = slice(DSH * hg, DSH * (hg + 1))
        wq_sh = np.asarray(wq, np.float32)[rows][perm]
        wk_sh = np.asarray(wk, np.float32)[rows][perm]
        wv_sh = np.asarray(wv, np.float32)[rows]
        per_group.append({
            "wqT": _wpack(np.ascontiguousarray(wq_sh.T).astype(BF16)),
            "wkT": _wpack(np.ascontiguousarray(wk_sh.T).astype(BF16)),
            "wvT": _wpack(np.ascontiguousarray(wv_sh.T).astype(BF16)),
            "woT": np.ascontiguousarray(
                np.asarray(wo, np.float32)[:, rows].T).astype(BF16),
            "bqp": np.ascontiguousarray(
                np.asarray(bq, np.float32)[rows][perm].reshape(HPC, 128).T),
            "bkp": np.ascontiguousarray(
                np.asarray(bk, np.float32)[rows][perm].reshape(HPC, 128).T),
        })

    in_maps = []
    for core in range(N_CORES):
        b, hg = core // 4, core % 4
        m = {"xT": xT[b], "cosp": cosp, "sinp": sinp, "m2t": m2t}
        m.update(per_group[hg])
        in_maps.append(m)
    return in_maps


def _mask_pattern(m2b):
    """Block keep flags + per-block leading-zero column offsets."""
    SQ_, NG = m2b.shape[0], m2b.shape[1]
    keep, scols = [], []
    for q4 in range(SQ_):
        krow, srow = [], []
        for kv in range(NG * 4):
            blk = m2b[q4, kv // 4, :, kv % 4]            # [128, 512]
            cols = np.nonzero(np.any(blk != 0, axis=0))[0]
            if cols.size == 0:
                krow.append(False)
                srow.append(0)
            else:
                krow.append(True)
                srow.append(int(cols[0]) // 128 * 128)
        first = next((kv for kv in range(NG * 4) if krow[kv]), None)
        if first is not None:
            srow[first] = 0   # chain head initializes full PSUM width
        keep.append(tuple(krow))
        scols.append(tuple(srow))
    return tuple(keep), tuple(scols)


def kernel(x, freqs, hard_mask, soft_mask, wq, bq, wk, bk, wv, bv, wo, bo,
           _trace=False, _tmpdir=None):
    s_len = x.shape[1]
    in_maps = prepare_inputs(x, freqs, hard_mask, soft_mask, wq, bq, wk, bk,
                             wv, wo, s_len=s_len)
    keep, scols = _mask_pattern(in_maps[0]["m2t"])
    ckey = (s_len, keep, scols)
    if ckey not in _NC_CACHE:
        _NC_CACHE[ckey] = build_bass(s_len, keep, scols)
    nc = _NC_CACHE[ckey]
    kwargs = {}
    if _trace:
        kwargs = {"trace": True, "tmpdir": _tmpdir}
    res = run_bass_kernel_spmd(nc, in_maps, core_ids=list(range(N_CORES)),
                               **kwargs)
    # bv folded out of the kernel: sum_kv attn == 1, so + bv per channel
    # post-softmax, and through wo that is the constant wo @ bv.
    bo_eff = (np.asarray(bo, np.float32)
              + np.asarray(wo, np.float32) @ np.asarray(bv, np.float32))
    out = np.empty((B, s_len, D), np.float32)
    for b in range(B):
        acc = res.results[4 * b]["y"].astype(np.float32)
        for hg in range(1, 4):
            acc = acc + res.results[4 * b + hg]["y"].astype(np.float32)
        out[b] = acc + bo_eff[None, :]
    kernel.last_result = res
    return out
